# revision 11
# baseline (speedup 1.0000x reference)
"""DLinear Trainium2 kernel (nn_DLinear_45990509805636).

Math: with T=17 and KERNEL_SIZE=37 (PAD=18), every moving-average window
covers the whole sequence plus replicated edges, so

    trend[b,t,:] = (S + (18-t)*x0 + (t+2)*x16) / 37,   S = sum_t x[:,t,:]
    out = x_t @ Ws[t] + trend_raw_t @ Wd[t] + bias[t],
    Wd = (Wt - Ws)/37 (host-folded), trend_raw_t = P + t*Q,
    P = S + 18*x0 + 2*x16, Q = x16 - x0.

v3 design (PE-bound problem; trace showed PE 99% busy mid-kernel):
  - No per-(t,j) K=1 bias matmuls: bias broadcast [128,T*D] is built once
    by 17 K=1 matmuls during the DMA-bound prologue and fused into the
    PSUM->SBUF drains (scalar_tensor_tensor adds it for free).
  - Phase B (trend @ Wd) contraction split: c-chunks 2:4 in bf16,
    c-chunks 0:2 as ONE fp8e4 DoubleRow matmul (K=256 virtual).
    trend/256 and Wd*256 make the DR product scale-1 so all 3 B matmuls
    share one PSUM group with phase A (post-prologue tokens: 7 MMs/group,
    single fused drain).  Host-sim rel err 1.60e-2 < 2e-2 gate.
  - Drains/joins alternate DVE <-> GpSimd (Pool was 0% busy in baseline);
    S-sum is also split by c-chunks across both engines.
  - Host layouts are partition-major [128, T, ...] so every DMA is >=4KB
    contiguous per partition; x rides 5 multi-token ~1-2MB dma_starts.

Sharding: data-parallel over batch, 8 cores x 512 rows; weights replicated.
"""

import os
import sys

sys.path.insert(0, "/opt/trn_rl_repo")

import numpy as np
import ml_dtypes

from concourse import bacc
import concourse.mybir as mybir
import concourse.tile as tile
from concourse.bass_utils import run_bass_kernel_spmd

dt = mybir.dt

B, T, C, D = 4096, 17, 512, 512
NCORES = 8
BC = B // NCORES          # 512 batch rows per core
KC = C // 128             # 4 contraction chunks
JB = BC // 128            # 4 output-row tiles per core
NDR = 2                   # c-chunks (0:NDR) through the fp8 DoubleRow MM
SCL = 256.0               # trend/SCL, Wd*SCL -> DR product is scale-1

MODE = os.environ.get("DLINEAR_MODE", "v3")
PRE_T = int(os.environ.get("DLINEAR_PRET", "9"))
POOL_DRAINS = os.environ.get("DLINEAR_POOL", "1") == "1"


def build_v3():
    idt = dt.bfloat16
    f8 = dt.float8e4
    NB = KC - NDR  # bf16 B chunks
    nc = bacc.Bacc(None, target_bir_lowering=False, name="dlinear_v5")
    xt = nc.dram_tensor("xt", [128, T, KC * BC], idt, kind="ExternalInput")
    wst = nc.dram_tensor("wst", [128, T, KC, D], idt, kind="ExternalInput")
    wdb = nc.dram_tensor("wdb", [128, T, NB, D], idt, kind="ExternalInput")
    wd8 = nc.dram_tensor("wd8", [128, T, NDR, D], f8, kind="ExternalInput")
    bias = nc.dram_tensor("bias", [1, T * D], idt, kind="ExternalInput")
    out = nc.dram_tensor("out", [BC, T, D], dt.float16, kind="ExternalOutput")

    with tile.TileContext(nc) as tc:
        with (
            tc.tile_pool(name="xres", bufs=1) as xres,
            tc.tile_pool(name="consts", bufs=1) as consts,
            tc.tile_pool(name="stats", bufs=1) as stats,
            tc.tile_pool(name="wsbuf", bufs=4) as wsbuf,
            tc.tile_pool(name="wdbuf", bufs=2) as wdbuf,
            tc.tile_pool(name="tbuf", bufs=3) as tbuf,
            tc.tile_pool(name="abuf", bufs=PRE_T * JB) as abuf,
            tc.tile_pool(name="obuf", bufs=6) as obuf,
            tc.tile_pool(name="dtmp", bufs=4) as dtmp,
            tc.tile_pool(name="psum_a", bufs=4, space="PSUM") as psum_a,
            tc.tile_pool(name="psum_b", bufs=3, space="PSUM") as psum_b,
            tc.tile_pool(name="psum_c", bufs=1, space="PSUM") as psum_c,
        ):
            drain_ctr = [0]

            def drain(out_ap, ps, addend):
                # PSUM->SBUF + elementwise add, split across engines:
                # DVE does it in one stt; the other half rides ACT (PSUM
                # copy) + Pool (SBUF add) to keep DVE off the critical
                # path.  (Pool cannot read PSUM; ACT cannot add.)
                drain_ctr[0] += 1
                if drain_ctr[0] % 2 == 0:
                    nc.vector.scalar_tensor_tensor(
                        out_ap, ps, 1.0, addend,
                        mybir.AluOpType.mult, mybir.AluOpType.add,
                    )
                else:
                    tmp = dtmp.tile([128, D], idt, tag="dtmp", name="dtmp")
                    nc.scalar.copy(tmp, ps)
                    nc.gpsimd.tensor_tensor(out_ap, tmp, addend,
                                            mybir.AluOpType.add)

            ones = consts.tile([1, 128], idt)
            nc.vector.memset(ones, 1.0)
            bsb = consts.tile([1, T * D], idt)
            nc.sync.dma_start(bsb, bias[:])
            bias_sb = consts.tile([128, T, D], idt)

            def build_bias(t):
                psc = psum_c.tile([128, D], dt.float32, tag="psc", name="psc")
                nc.tensor.matmul(psc, ones, bsb[:, t * D:(t + 1) * D],
                                 start=True, stop=True)
                nc.scalar.copy(bias_sb[:, t], psc)

            xsb = xres.tile([128, T, KC * BC], idt)

            ws_tiles = {}

            def load_ws(t):
                wss = wsbuf.tile([128, KC, D], idt, tag="ws", name="ws")
                nc.sync.dma_start(wss, wst[:, t])
                ws_tiles[t] = wss

            wd_tiles = {}

            def load_wd(tp):  # token pair (tp, tp+1) within one stream
                lim = PRE_T if tp < PRE_T else T
                nt = min(2, lim - tp)
                wdbt = wdbuf.tile([128, nt, NB, D], idt, tag="wdb", name="wdb")
                nc.sync.dma_start(wdbt, wdb[:, tp:tp + nt])
                wd8t = wdbuf.tile([128, nt, NDR, D], f8, tag="wd8", name="wd8")
                nc.sync.dma_start(wd8t, wd8[:, tp:tp + nt])
                for i in range(nt):
                    wd_tiles[tp + i] = (wdbt, wd8t, i)

            def emit_a_mms(ps, t, j, stop):
                wss = ws_tiles[t]
                for k in range(KC):
                    nc.tensor.matmul(
                        ps, xsb[:, t, k * BC + j * 128:k * BC + (j + 1) * 128],
                        wss[:, k],
                        start=(k == 0), stop=(stop and k == KC - 1),
                    )

            def emit_b_mms(ps, t, j, tr, start):
                tbf, t8 = tr
                wdbt, wd8t, wi = wd_tiles[t]
                for ci in range(NB):
                    c0 = (NDR + ci) * BC + j * 128
                    nc.tensor.matmul(
                        ps, tbf[:, c0:c0 + 128],
                        wdbt[:, wi, ci],
                        start=(start and ci == 0), stop=False,
                    )
                nc.tensor.matmul(
                    ps, t8[:, :, j * 128:(j + 1) * 128], wd8t[:, wi],
                    start=False, stop=True,
                    perf_mode=mybir.MatmulPerfMode.DoubleRow,
                )

            # ---- prologue: stream x + ws, run phase A, build bias ----
            x_groups = {0: (0, 3), 1: (3, 6), 3: (6, 10), 5: (10, 14),
                        7: (14, 17)}
            outa = {}
            for t in range(PRE_T):
                load_ws(t)
                if t in x_groups:
                    a, b = x_groups[t]
                    nc.sync.dma_start(xsb[:, a:b], xt[:, a:b])
                for bt in (t, t + PRE_T):
                    if bt < T:
                        build_bias(bt)
                if t == PRE_T - 2:
                    load_wd(0)
                if t == PRE_T - 1:
                    load_wd(PRE_T)
                for j in range(JB):
                    psa = psum_a.tile([128, D], dt.float32, tag="psa", name="psa")
                    emit_a_mms(psa, t, j, stop=True)
                    oa = abuf.tile([128, D], idt, tag="outa", name="outa")
                    drain(oa, psa, bias_sb[:, t])
                    outa[(t, j)] = oa
            for t in range(PRE_T, PRE_T + 2):
                load_ws(t)

            # ---- stats: S,P,Q as full-width tensor_tensor/tensor_scalar
            # ops (2x/4x DVE modes; scalar_tensor_tensor is always 1x) ----
            S = stats.tile([128, KC * BC], idt)
            Q = stats.tile([128, KC * BC], idt)
            nc.vector.tensor_tensor(S[:], xsb[:, 0], xsb[:, 1],
                                    mybir.AluOpType.add)
            for t in range(2, T):
                nc.vector.tensor_tensor(S[:], S[:], xsb[:, t],
                                        mybir.AluOpType.add)
            nc.vector.tensor_tensor(Q[:], xsb[:, 16], xsb[:, 0],
                                    mybir.AluOpType.subtract)
            P = S  # P overwrites S in place (S unused afterwards)
            nc.vector.scalar_tensor_tensor(P[:], xsb[:, 0], 18.0, S[:],
                                           mybir.AluOpType.mult, mybir.AluOpType.add)
            nc.vector.scalar_tensor_tensor(P[:], xsb[:, 16], 2.0, P[:],
                                           mybir.AluOpType.mult, mybir.AluOpType.add)

            # trend: TWO parallel iterative bf16 chains (seeds t=0 copy,
            # t=LH stt) -- halves both chain latency and accumulated
            # rounding.  fp8 DR operand = ACT copy with scale=1/SCL.
            LH = PRE_T  # heavy-stream base token
            chain_prev = {}

            def make_trend(t):
                tr = tbuf.tile([128, KC * BC], idt, tag="tbf", name="tbf")
                if t == 0:
                    nc.vector.tensor_copy(tr[:], P[:])
                elif t == LH:
                    nc.vector.scalar_tensor_tensor(
                        tr[:], Q[:], float(t), P[:],
                        mybir.AluOpType.mult, mybir.AluOpType.add,
                    )
                else:
                    nc.vector.tensor_tensor(tr[:], chain_prev[t - 1][:], Q[:],
                                            mybir.AluOpType.add)
                chain_prev[t] = tr
                t8 = tbuf.tile([128, NDR, BC], f8, tag="t8", name="t8")
                nc.scalar.mul(
                    t8[:],
                    tr[:, 0:NDR * BC].rearrange("p (k b) -> p k b", k=NDR),
                    1.0 / SCL,
                )
                return (tr, t8)

            # ---- main loop: interleave light (B-only, tokens <LH) and
            # heavy (A+B, tokens >=LH) groups to keep PE dense ----
            trend_tiles = {t: make_trend(t) for t in (0, LH)}

            def store_out(t, j, osb):
                nc.scalar.dma_start(out[j * 128:(j + 1) * 128, t:t + 1, :], osb)

            def emit_light(t, js):
                tr = trend_tiles[t]
                for j in js:
                    psb = psum_b.tile([128, D], dt.float32, tag="psb",
                                      name="psb")
                    emit_b_mms(psb, t, j, tr, start=True)
                    osb = obuf.tile([128, 1, D], dt.float16, tag="osb",
                                    name="osb")
                    drain(osb[:, 0], psb, outa.pop((t, j)))
                    store_out(t, j, osb)

            def emit_heavy(t, js):
                tr = trend_tiles[t]
                for j in js:
                    psa = psum_a.tile([128, D], dt.float32, tag="psa",
                                      name="psa")
                    emit_a_mms(psa, t, j, stop=False)
                    emit_b_mms(psa, t, j, tr, start=False)
                    osb = obuf.tile([128, 1, D], dt.float16, tag="osb",
                                    name="osb")
                    drain(osb[:, 0], psa, bias_sb[:, t])
                    store_out(t, j, osb)

            for s in range(LH):
                heavy = LH + s
                # lookahead: trend one step ahead per chain; ws/wd streams
                if s + 1 < LH:
                    trend_tiles[s + 1] = make_trend(s + 1)
                if heavy + 1 < T:
                    trend_tiles[heavy + 1] = make_trend(heavy + 1)
                if heavy + 2 < T:
                    load_ws(heavy + 2)
                if s % 2 == 0:
                    if s + 2 < LH:
                        load_wd(s + 2)
                    if heavy + 2 < T:
                        load_wd(heavy + 2)
                has_heavy = heavy < T
                emit_light(s, (0, 1))
                if has_heavy:
                    emit_heavy(heavy, (0, 1))
                emit_light(s, (2, 3))
                if has_heavy:
                    emit_heavy(heavy, (2, 3))
                trend_tiles.pop(s)
                if has_heavy:
                    trend_tiles.pop(heavy)
    nc.compile()
    return nc


def build_bf16():
    # previous-generation kernel (kept for A/B testing via DLINEAR_MODE=bf16)
    idt = dt.bfloat16
    nc = bacc.Bacc(None, target_bir_lowering=False, name="dlinear_bf16")
    xt = nc.dram_tensor("xt", [T, 128, KC, BC], idt, kind="ExternalInput")
    wst = nc.dram_tensor("wst", [T, 128, KC, D], idt, kind="ExternalInput")
    wdt = nc.dram_tensor("wdt", [T, 128, KC, D], idt, kind="ExternalInput")
    bias = nc.dram_tensor("bias", [1, T * D], idt, kind="ExternalInput")
    out = nc.dram_tensor("out", [BC, T, D], dt.float16, kind="ExternalOutput")

    with tile.TileContext(nc) as tc:
        with (
            tc.tile_pool(name="xres", bufs=1) as xres,
            tc.tile_pool(name="consts", bufs=1) as consts,
            tc.tile_pool(name="stats", bufs=1) as stats,
            tc.tile_pool(name="wsbuf", bufs=4) as wsbuf,
            tc.tile_pool(name="wdbuf", bufs=2) as wdbuf,
            tc.tile_pool(name="tbuf", bufs=2) as tbuf,
            tc.tile_pool(name="abuf", bufs=44) as abuf,
            tc.tile_pool(name="obuf", bufs=6) as obuf,
            tc.tile_pool(name="dtmp", bufs=6) as dtmp,
            tc.tile_pool(name="psum_a", bufs=4, space="PSUM") as psum_a,
            tc.tile_pool(name="psum_b", bufs=4, space="PSUM") as psum_b,
        ):
            ones = consts.tile([1, 128], idt)
            nc.vector.memset(ones, 1.0)
            bsb = consts.tile([1, T * D], idt)
            nc.sync.dma_start(bsb, bias[:])

            xsb = xres.tile([128, T, KC, BC], idt)

            def emit_phase_a(t, wss, j):
                psa = psum_a.tile([128, D], dt.float32, tag="psa", name="psa")
                nc.tensor.matmul(psa, ones, bsb[:, t * D:(t + 1) * D],
                                 start=True, stop=False)
                for k in range(KC):
                    nc.tensor.matmul(
                        psa, xsb[:, t, k, j * 128:(j + 1) * 128], wss[:, k],
                        start=False, stop=(k == KC - 1),
                    )
                outa = abuf.tile([128, D], idt, tag="outa", name="outa")
                nc.scalar.copy(outa, psa)
                return outa

            PRE = 12
            ws_pre = {}
            for t in range(4):
                wss = wsbuf.tile([128, KC, D], idt, tag="ws", name="ws")
                nc.sync.dma_start(wss[:, 0:2], wst[t, :, 0:2])
                nc.sync.dma_start(wss[:, 2:4], wst[t, :, 2:4])
                ws_pre[t] = wss
            for t in range(T):
                nc.sync.dma_start(xsb[:, t, 0:2], xt[t, :, 0:2])
                nc.sync.dma_start(xsb[:, t, 2:4], xt[t, :, 2:4])
            wd_pre = {}
            for t in range(3):
                wds = wdbuf.tile([128, KC, D], idt, tag="wd", name="wd")
                nc.sync.dma_start(wds[:, 0:2], wdt[t, :, 0:2])
                nc.sync.dma_start(wds[:, 2:4], wdt[t, :, 2:4])
                wd_pre[t] = wds
            for t in range(4, PRE):
                wss = wsbuf.tile([128, KC, D], idt, tag="ws", name="ws")
                nc.sync.dma_start(wss[:, 0:2], wst[t, :, 0:2])
                nc.sync.dma_start(wss[:, 2:4], wst[t, :, 2:4])
                ws_pre[t] = wss

            outa_pre = {}
            for t in range(PRE):
                for j in range(JB):
                    outa_pre[(t, j)] = emit_phase_a(t, ws_pre[t], j)

            S = stats.tile([128, KC, BC], idt)
            P = stats.tile([128, KC, BC], idt)
            Q = stats.tile([128, KC, BC], idt)
            nc.vector.tensor_tensor(S[:], xsb[:, 0], xsb[:, 1], mybir.AluOpType.add)
            for t in range(2, T):
                nc.vector.tensor_tensor(S[:], S[:], xsb[:, t], mybir.AluOpType.add)
            nc.vector.scalar_tensor_tensor(P[:], xsb[:, 0], 18.0, S[:],
                                           mybir.AluOpType.mult, mybir.AluOpType.add)
            nc.vector.scalar_tensor_tensor(P[:], xsb[:, 16], 2.0, P[:],
                                           mybir.AluOpType.mult, mybir.AluOpType.add)
            nc.vector.tensor_tensor(Q[:], xsb[:, 16], xsb[:, 0], mybir.AluOpType.subtract)

            osb_cur = {}
            for t in range(T):
                if t >= PRE:
                    wss = wsbuf.tile([128, KC, D], idt, tag="ws", name="ws")
                    nc.sync.dma_start(wss[:, 0:2], wst[t, :, 0:2])
                    nc.sync.dma_start(wss[:, 2:4], wst[t, :, 2:4])
                    ws_pre[t] = wss
                if t < 3:
                    wds = wd_pre.pop(t)
                else:
                    wds = wdbuf.tile([128, KC, D], idt, tag="wd", name="wd")
                    nc.sync.dma_start(wds[:, 0:2], wdt[t, :, 0:2])
                    nc.sync.dma_start(wds[:, 2:4], wdt[t, :, 2:4])

                trend = tbuf.tile([128, KC, BC], idt, tag="trend", name="trend")
                if t == 0:
                    nc.vector.tensor_copy(trend[:], P[:])
                else:
                    nc.vector.tensor_scalar_mul(trend[:], Q[:], float(t))
                    nc.vector.tensor_tensor(trend[:], trend[:], P[:], mybir.AluOpType.add)

                for j in range(JB):
                    if t < PRE:
                        outa = outa_pre.pop((t, j))
                    else:
                        outa = emit_phase_a(t, ws_pre[t], j)
                    psb = psum_b.tile([128, D], dt.float32, tag="psb", name="psb")
                    for k in range(KC):
                        nc.tensor.matmul(
                            psb, trend[:, k, j * 128:(j + 1) * 128], wds[:, k],
                            start=(k == 0), stop=(k == KC - 1),
                        )
                    if t == T - 1:
                        osb = obuf.tile([128, 1, D], dt.float16, tag="osb1", name="osb1")
                        nc.vector.scalar_tensor_tensor(
                            osb[:, 0], psb, 1.0, outa,
                            mybir.AluOpType.mult, mybir.AluOpType.add,
                        )
                        nc.scalar.dma_start(
                            out[j * 128:(j + 1) * 128, t:t + 1, :], osb)
                    else:
                        if t % 2 == 0:
                            osb = obuf.tile([128, 2, D], dt.float16, tag="osb", name="osb")
                            osb_cur[j] = osb
                        else:
                            osb = osb_cur[j]
                        nc.vector.scalar_tensor_tensor(
                            osb[:, t % 2], psb, 1.0, outa,
                            mybir.AluOpType.mult, mybir.AluOpType.add,
                        )
                        if t % 2 == 1:
                            nc.scalar.dma_start(
                                out[j * 128:(j + 1) * 128, t - 1:t + 1, :], osb)
    nc.compile()
    return nc


_NC_CACHE = {}


def _get_nc(mode):
    if mode not in _NC_CACHE:
        _NC_CACHE[mode] = build_v3() if mode == "v3" else build_bf16()
    return _NC_CACHE[mode]


def _to_pkt(w):  # [T, D, C] -> [128, T, KC, D]  (c = k*128 + p)
    wt = w.transpose(2, 0, 1).reshape(KC, 128, T, D)
    return np.ascontiguousarray(wt.transpose(1, 2, 0, 3))


def kernel(x, W_seasonal, b_seasonal, W_trend, b_trend, _trace=False):
    mode = MODE
    bf16 = ml_dtypes.bfloat16
    e4 = ml_dtypes.float8_e4m3
    nc = _get_nc(mode)

    bias = (b_seasonal + b_trend).reshape(1, T * D).astype(bf16)
    Wd = (W_trend - W_seasonal) / 37.0

    if mode == "v3":
        wst = _to_pkt(W_seasonal).astype(bf16)
        wdt = _to_pkt(Wd)
        wdb = np.ascontiguousarray(wdt[:, :, NDR:KC]).astype(bf16)
        wd8 = np.ascontiguousarray(wdt[:, :, 0:NDR] * SCL).astype(e4)
        in_maps = []
        for i in range(NCORES):
            xs = x[i * BC:(i + 1) * BC]                    # [BC, T, C]
            xti = xs.transpose(2, 1, 0).reshape(KC, 128, T, BC)
            xti = np.ascontiguousarray(
                xti.transpose(1, 2, 0, 3)).astype(bf16).reshape(128, T, KC * BC)
            in_maps.append({"xt": xti, "wst": wst, "wdb": wdb, "wd8": wd8,
                            "bias": bias})
    else:
        def to_tpkd(w):  # [T, D, C] -> [T, 128, KC, D]
            wt = w.transpose(0, 2, 1).reshape(T, KC, 128, D)
            return np.ascontiguousarray(wt.transpose(0, 2, 1, 3))

        wst = to_tpkd(W_seasonal).astype(bf16)
        wdt = to_tpkd(Wd).astype(bf16)
        in_maps = []
        for i in range(NCORES):
            xs = x[i * BC:(i + 1) * BC]
            xti = xs.transpose(1, 2, 0).reshape(T, KC, 128, BC)
            xti = np.ascontiguousarray(xti.transpose(0, 2, 1, 3)).astype(bf16)
            in_maps.append({"xt": xti, "wst": wst, "wdt": wdt, "bias": bias})

    res = run_bass_kernel_spmd(
        nc, in_maps, core_ids=list(range(NCORES)), trace=_trace
    )
    outp = np.concatenate([r["out"] for r in res.results], axis=0)
    if outp.dtype != np.float32:
        outp = outp.astype(np.float32)
    if _trace:
        return outp, res
    return outp


if __name__ == "__main__":
    rng = np.random.default_rng(0)
    x = rng.standard_normal((B, T, C), dtype=np.float32)
    Ws = rng.uniform(-0.04, 0.04, (T, D, C)).astype(np.float32)
    Wt = rng.uniform(-0.04, 0.04, (T, D, C)).astype(np.float32)
    bs = rng.uniform(-0.04, 0.04, (T, D)).astype(np.float32)
    bt = rng.uniform(-0.04, 0.04, (T, D)).astype(np.float32)
    o = kernel(x, Ws, bs, Wt, bt)
    print("out shape:", o.shape, o.dtype)


# revision 12
# speedup vs baseline: 1.0762x; 1.0762x over previous
"""DLinear Trainium2 kernel (nn_DLinear_45990509805636).

Math: with T=17 and KERNEL_SIZE=37 (PAD=18), every moving-average window
covers the whole sequence plus replicated edges, so

    trend[b,t,:] = (S + (18-t)*x0 + (t+2)*x16) / 37,   S = sum_t x[:,t,:]
    out = x_t @ Ws[t] + trend_raw_t @ Wd[t] + bias[t],
    Wd = (Wt - Ws)/37 (host-folded), trend_raw_t = P + t*Q,
    P = S + 18*x0 + 2*x16, Q = x16 - x0.

v3 design (PE-bound problem; trace showed PE 99% busy mid-kernel):
  - No per-(t,j) K=1 bias matmuls: bias broadcast [128,T*D] is built once
    by 17 K=1 matmuls during the DMA-bound prologue and fused into the
    PSUM->SBUF drains (scalar_tensor_tensor adds it for free).
  - Phase B (trend @ Wd) contraction split: c-chunks 2:4 in bf16,
    c-chunks 0:2 as ONE fp8e4 DoubleRow matmul (K=256 virtual).
    trend/256 and Wd*256 make the DR product scale-1 so all 3 B matmuls
    share one PSUM group with phase A (post-prologue tokens: 7 MMs/group,
    single fused drain).  Host-sim rel err 1.60e-2 < 2e-2 gate.
  - Drains/joins alternate DVE <-> GpSimd (Pool was 0% busy in baseline);
    S-sum is also split by c-chunks across both engines.
  - Host layouts are partition-major [128, T, ...] so every DMA is >=4KB
    contiguous per partition; x rides 5 multi-token ~1-2MB dma_starts.

Sharding: data-parallel over batch, 8 cores x 512 rows; weights replicated.
"""

import os
import sys

sys.path.insert(0, "/opt/trn_rl_repo")

import numpy as np
import ml_dtypes

from concourse import bacc
import concourse.mybir as mybir
import concourse.tile as tile
from concourse.bass_utils import run_bass_kernel_spmd

dt = mybir.dt

B, T, C, D = 4096, 17, 512, 512
NCORES = 8
BC = B // NCORES          # 512 batch rows per core
KC = C // 128             # 4 contraction chunks
JB = BC // 128            # 4 output-row tiles per core
NDR = 2                   # c-chunks (0:NDR) through the fp8 DoubleRow MM
SCL = 256.0               # trend/SCL, Wd*SCL -> DR product is scale-1

MODE = os.environ.get("DLINEAR_MODE", "v3")
PRE_T = int(os.environ.get("DLINEAR_PRET", "9"))
POOL_DRAINS = os.environ.get("DLINEAR_POOL", "1") == "1"


def build_v3():
    idt = dt.bfloat16
    f8 = dt.float8e4
    NB = KC - NDR  # bf16 B chunks
    nc = bacc.Bacc(None, target_bir_lowering=False, name="dlinear_v5")
    xt = nc.dram_tensor("xt", [128, T, KC * BC], idt, kind="ExternalInput")
    wst = nc.dram_tensor("wst", [128, T, KC, D], idt, kind="ExternalInput")
    wdb = nc.dram_tensor("wdb", [128, T, NB, D], idt, kind="ExternalInput")
    wd8 = nc.dram_tensor("wd8", [128, T, NDR, D], f8, kind="ExternalInput")
    bias = nc.dram_tensor("bias", [1, T * D], idt, kind="ExternalInput")
    out = nc.dram_tensor("out", [BC, T, D], dt.float16, kind="ExternalOutput")

    with tile.TileContext(nc) as tc:
        with (
            tc.tile_pool(name="xres", bufs=1) as xres,
            tc.tile_pool(name="consts", bufs=1) as consts,
            tc.tile_pool(name="stats", bufs=1) as stats,
            tc.tile_pool(name="wsbuf", bufs=4) as wsbuf,
            tc.tile_pool(name="wdbuf", bufs=6) as wdbuf,
            tc.tile_pool(name="tbuf", bufs=3) as tbuf,
            tc.tile_pool(name="abuf", bufs=PRE_T * JB) as abuf,
            tc.tile_pool(name="obuf", bufs=6) as obuf,
            tc.tile_pool(name="dtmp", bufs=4) as dtmp,
            tc.tile_pool(name="psum_a", bufs=4, space="PSUM") as psum_a,
            tc.tile_pool(name="psum_b", bufs=3, space="PSUM") as psum_b,
            tc.tile_pool(name="psum_c", bufs=1, space="PSUM") as psum_c,
        ):
            drain_ctr = [0]

            def drain(out_ap, ps, addend):
                # PSUM->SBUF + elementwise add, split across engines:
                # DVE does it in one stt; the other half rides ACT (PSUM
                # copy) + Pool (SBUF add) to keep DVE off the critical
                # path.  (Pool cannot read PSUM; ACT cannot add.)
                drain_ctr[0] += 1
                if drain_ctr[0] % 2 == 0:
                    nc.vector.scalar_tensor_tensor(
                        out_ap, ps, 1.0, addend,
                        mybir.AluOpType.mult, mybir.AluOpType.add,
                    )
                else:
                    tmp = dtmp.tile([128, D], idt, tag="dtmp", name="dtmp")
                    nc.scalar.copy(tmp, ps)
                    nc.gpsimd.tensor_tensor(out_ap, tmp, addend,
                                            mybir.AluOpType.add)

            ones = consts.tile([1, 128], idt)
            nc.vector.memset(ones, 1.0)
            bsb = consts.tile([1, T * D], idt)
            nc.sync.dma_start(bsb, bias[:])
            bias_sb = consts.tile([128, T, D], idt)

            def build_bias(t):
                psc = psum_c.tile([128, D], dt.float32, tag="psc", name="psc")
                nc.tensor.matmul(psc, ones, bsb[:, t * D:(t + 1) * D],
                                 start=True, stop=True)
                nc.scalar.copy(bias_sb[:, t], psc)

            xsb = xres.tile([128, T, KC * BC], idt)

            ws_tiles = {}

            def load_ws(t):
                wss = wsbuf.tile([128, KC, D], idt, tag="ws", name="ws")
                nc.sync.dma_start(wss, wst[:, t])
                ws_tiles[t] = wss

            wd_tiles = {}

            def load_wd(tp):  # single token (interleaved streams need
                # 4+ concurrently-live pairs; singles keep the pool small)
                wdbt = wdbuf.tile([128, 1, NB, D], idt, tag="wdb", name="wdb")
                nc.sync.dma_start(wdbt, wdb[:, tp:tp + 1])
                wd8t = wdbuf.tile([128, 1, NDR, D], f8, tag="wd8", name="wd8")
                nc.sync.dma_start(wd8t, wd8[:, tp:tp + 1])
                wd_tiles[tp] = (wdbt, wd8t, 0)

            def emit_a_mms(ps, t, j, stop):
                wss = ws_tiles[t]
                for k in range(KC):
                    nc.tensor.matmul(
                        ps, xsb[:, t, k * BC + j * 128:k * BC + (j + 1) * 128],
                        wss[:, k],
                        start=(k == 0), stop=(stop and k == KC - 1),
                    )

            def emit_b_mms(ps, t, j, tr, start):
                tbf, t8 = tr
                wdbt, wd8t, wi = wd_tiles[t]
                for ci in range(NB):
                    c0 = (NDR + ci) * BC + j * 128
                    nc.tensor.matmul(
                        ps, tbf[:, c0:c0 + 128],
                        wdbt[:, wi, ci],
                        start=(start and ci == 0), stop=False,
                    )
                nc.tensor.matmul(
                    ps, t8[:, :, j * 128:(j + 1) * 128], wd8t[:, wi],
                    start=False, stop=True,
                    perf_mode=mybir.MatmulPerfMode.DoubleRow,
                )

            # ---- prologue: stream x + ws, run phase A, build bias ----
            x_groups = {0: (0, 3), 1: (3, 6), 3: (6, 10), 5: (10, 14),
                        7: (14, 17)}
            outa = {}
            for t in range(PRE_T):
                load_ws(t)
                if t in x_groups:
                    a, b = x_groups[t]
                    nc.sync.dma_start(xsb[:, a:b], xt[:, a:b])
                for bt in (t, t + PRE_T):
                    if bt < T:
                        build_bias(bt)
                if t == PRE_T - 2:
                    load_wd(0)
                    load_wd(1)
                if t == PRE_T - 1:
                    load_wd(PRE_T)
                    load_wd(PRE_T + 1)
                for j in range(JB):
                    psa = psum_a.tile([128, D], dt.float32, tag="psa", name="psa")
                    emit_a_mms(psa, t, j, stop=True)
                    oa = abuf.tile([128, D], idt, tag="outa", name="outa")
                    drain(oa, psa, bias_sb[:, t])
                    outa[(t, j)] = oa
            for t in range(PRE_T, PRE_T + 2):
                load_ws(t)

            # ---- stats: S,P,Q as full-width tensor_tensor/tensor_scalar
            # ops (2x/4x DVE modes; scalar_tensor_tensor is always 1x) ----
            S = stats.tile([128, KC * BC], idt)
            Q = stats.tile([128, KC * BC], idt)
            nc.vector.tensor_tensor(S[:], xsb[:, 0], xsb[:, 1],
                                    mybir.AluOpType.add)
            for t in range(2, T):
                nc.vector.tensor_tensor(S[:], S[:], xsb[:, t],
                                        mybir.AluOpType.add)
            nc.vector.tensor_tensor(Q[:], xsb[:, 16], xsb[:, 0],
                                    mybir.AluOpType.subtract)
            P = S  # P overwrites S in place (S unused afterwards)
            nc.vector.scalar_tensor_tensor(P[:], xsb[:, 0], 18.0, S[:],
                                           mybir.AluOpType.mult, mybir.AluOpType.add)
            nc.vector.scalar_tensor_tensor(P[:], xsb[:, 16], 2.0, P[:],
                                           mybir.AluOpType.mult, mybir.AluOpType.add)

            # trend: TWO parallel iterative bf16 chains (seeds t=0 copy,
            # t=LH stt) -- halves both chain latency and accumulated
            # rounding.  fp8 DR operand = ACT copy with scale=1/SCL.
            LH = PRE_T  # heavy-stream base token
            chain_prev = {}

            def make_trend(t):
                tr = tbuf.tile([128, KC * BC], idt, tag="tbf", name="tbf")
                if t == 0:
                    nc.vector.tensor_copy(tr[:], P[:])
                elif t == LH:
                    nc.vector.scalar_tensor_tensor(
                        tr[:], Q[:], float(t), P[:],
                        mybir.AluOpType.mult, mybir.AluOpType.add,
                    )
                else:
                    nc.vector.tensor_tensor(tr[:], chain_prev[t - 1][:], Q[:],
                                            mybir.AluOpType.add)
                chain_prev[t] = tr
                t8 = tbuf.tile([128, NDR, BC], f8, tag="t8", name="t8")
                nc.scalar.mul(
                    t8[:],
                    tr[:, 0:NDR * BC].rearrange("p (k b) -> p k b", k=NDR),
                    1.0 / SCL,
                )
                return (tr, t8)

            # ---- main loop: interleave light (B-only, tokens <LH) and
            # heavy (A+B, tokens >=LH) groups to keep PE dense ----
            trend_tiles = {t: make_trend(t) for t in (0, LH)}

            def store_out(t, j, osb):
                nc.scalar.dma_start(out[j * 128:(j + 1) * 128, t:t + 1, :], osb)

            def emit_light(t, js):
                tr = trend_tiles[t]
                for j in js:
                    psb = psum_b.tile([128, D], dt.float32, tag="psb",
                                      name="psb")
                    emit_b_mms(psb, t, j, tr, start=True)
                    osb = obuf.tile([128, 1, D], dt.float16, tag="osb",
                                    name="osb")
                    drain(osb[:, 0], psb, outa.pop((t, j)))
                    store_out(t, j, osb)

            def emit_heavy(t, js):
                tr = trend_tiles[t]
                for j in js:
                    psa = psum_a.tile([128, D], dt.float32, tag="psa",
                                      name="psa")
                    emit_a_mms(psa, t, j, stop=False)
                    emit_b_mms(psa, t, j, tr, start=False)
                    osb = obuf.tile([128, 1, D], dt.float16, tag="osb",
                                    name="osb")
                    drain(osb[:, 0], psa, bias_sb[:, t])
                    store_out(t, j, osb)

            for s in range(LH):
                heavy = LH + s
                # lookahead: trend one step ahead per chain; ws/wd streams
                if s + 1 < LH:
                    trend_tiles[s + 1] = make_trend(s + 1)
                if heavy + 1 < T:
                    trend_tiles[heavy + 1] = make_trend(heavy + 1)
                if heavy + 2 < T:
                    load_ws(heavy + 2)
                if s + 2 < LH:
                    load_wd(s + 2)
                if heavy + 2 < T:
                    load_wd(heavy + 2)
                has_heavy = heavy < T
                emit_light(s, (0, 1))
                if has_heavy:
                    emit_heavy(heavy, (0, 1))
                emit_light(s, (2, 3))
                if has_heavy:
                    emit_heavy(heavy, (2, 3))
                trend_tiles.pop(s)
                if has_heavy:
                    trend_tiles.pop(heavy)
    nc.compile()
    return nc


def build_bf16():
    # previous-generation kernel (kept for A/B testing via DLINEAR_MODE=bf16)
    idt = dt.bfloat16
    nc = bacc.Bacc(None, target_bir_lowering=False, name="dlinear_bf16")
    xt = nc.dram_tensor("xt", [T, 128, KC, BC], idt, kind="ExternalInput")
    wst = nc.dram_tensor("wst", [T, 128, KC, D], idt, kind="ExternalInput")
    wdt = nc.dram_tensor("wdt", [T, 128, KC, D], idt, kind="ExternalInput")
    bias = nc.dram_tensor("bias", [1, T * D], idt, kind="ExternalInput")
    out = nc.dram_tensor("out", [BC, T, D], dt.float16, kind="ExternalOutput")

    with tile.TileContext(nc) as tc:
        with (
            tc.tile_pool(name="xres", bufs=1) as xres,
            tc.tile_pool(name="consts", bufs=1) as consts,
            tc.tile_pool(name="stats", bufs=1) as stats,
            tc.tile_pool(name="wsbuf", bufs=4) as wsbuf,
            tc.tile_pool(name="wdbuf", bufs=6) as wdbuf,
            tc.tile_pool(name="tbuf", bufs=2) as tbuf,
            tc.tile_pool(name="abuf", bufs=44) as abuf,
            tc.tile_pool(name="obuf", bufs=6) as obuf,
            tc.tile_pool(name="dtmp", bufs=6) as dtmp,
            tc.tile_pool(name="psum_a", bufs=4, space="PSUM") as psum_a,
            tc.tile_pool(name="psum_b", bufs=4, space="PSUM") as psum_b,
        ):
            ones = consts.tile([1, 128], idt)
            nc.vector.memset(ones, 1.0)
            bsb = consts.tile([1, T * D], idt)
            nc.sync.dma_start(bsb, bias[:])

            xsb = xres.tile([128, T, KC, BC], idt)

            def emit_phase_a(t, wss, j):
                psa = psum_a.tile([128, D], dt.float32, tag="psa", name="psa")
                nc.tensor.matmul(psa, ones, bsb[:, t * D:(t + 1) * D],
                                 start=True, stop=False)
                for k in range(KC):
                    nc.tensor.matmul(
                        psa, xsb[:, t, k, j * 128:(j + 1) * 128], wss[:, k],
                        start=False, stop=(k == KC - 1),
                    )
                outa = abuf.tile([128, D], idt, tag="outa", name="outa")
                nc.scalar.copy(outa, psa)
                return outa

            PRE = 12
            ws_pre = {}
            for t in range(4):
                wss = wsbuf.tile([128, KC, D], idt, tag="ws", name="ws")
                nc.sync.dma_start(wss[:, 0:2], wst[t, :, 0:2])
                nc.sync.dma_start(wss[:, 2:4], wst[t, :, 2:4])
                ws_pre[t] = wss
            for t in range(T):
                nc.sync.dma_start(xsb[:, t, 0:2], xt[t, :, 0:2])
                nc.sync.dma_start(xsb[:, t, 2:4], xt[t, :, 2:4])
            wd_pre = {}
            for t in range(3):
                wds = wdbuf.tile([128, KC, D], idt, tag="wd", name="wd")
                nc.sync.dma_start(wds[:, 0:2], wdt[t, :, 0:2])
                nc.sync.dma_start(wds[:, 2:4], wdt[t, :, 2:4])
                wd_pre[t] = wds
            for t in range(4, PRE):
                wss = wsbuf.tile([128, KC, D], idt, tag="ws", name="ws")
                nc.sync.dma_start(wss[:, 0:2], wst[t, :, 0:2])
                nc.sync.dma_start(wss[:, 2:4], wst[t, :, 2:4])
                ws_pre[t] = wss

            outa_pre = {}
            for t in range(PRE):
                for j in range(JB):
                    outa_pre[(t, j)] = emit_phase_a(t, ws_pre[t], j)

            S = stats.tile([128, KC, BC], idt)
            P = stats.tile([128, KC, BC], idt)
            Q = stats.tile([128, KC, BC], idt)
            nc.vector.tensor_tensor(S[:], xsb[:, 0], xsb[:, 1], mybir.AluOpType.add)
            for t in range(2, T):
                nc.vector.tensor_tensor(S[:], S[:], xsb[:, t], mybir.AluOpType.add)
            nc.vector.scalar_tensor_tensor(P[:], xsb[:, 0], 18.0, S[:],
                                           mybir.AluOpType.mult, mybir.AluOpType.add)
            nc.vector.scalar_tensor_tensor(P[:], xsb[:, 16], 2.0, P[:],
                                           mybir.AluOpType.mult, mybir.AluOpType.add)
            nc.vector.tensor_tensor(Q[:], xsb[:, 16], xsb[:, 0], mybir.AluOpType.subtract)

            osb_cur = {}
            for t in range(T):
                if t >= PRE:
                    wss = wsbuf.tile([128, KC, D], idt, tag="ws", name="ws")
                    nc.sync.dma_start(wss[:, 0:2], wst[t, :, 0:2])
                    nc.sync.dma_start(wss[:, 2:4], wst[t, :, 2:4])
                    ws_pre[t] = wss
                if t < 3:
                    wds = wd_pre.pop(t)
                else:
                    wds = wdbuf.tile([128, KC, D], idt, tag="wd", name="wd")
                    nc.sync.dma_start(wds[:, 0:2], wdt[t, :, 0:2])
                    nc.sync.dma_start(wds[:, 2:4], wdt[t, :, 2:4])

                trend = tbuf.tile([128, KC, BC], idt, tag="trend", name="trend")
                if t == 0:
                    nc.vector.tensor_copy(trend[:], P[:])
                else:
                    nc.vector.tensor_scalar_mul(trend[:], Q[:], float(t))
                    nc.vector.tensor_tensor(trend[:], trend[:], P[:], mybir.AluOpType.add)

                for j in range(JB):
                    if t < PRE:
                        outa = outa_pre.pop((t, j))
                    else:
                        outa = emit_phase_a(t, ws_pre[t], j)
                    psb = psum_b.tile([128, D], dt.float32, tag="psb", name="psb")
                    for k in range(KC):
                        nc.tensor.matmul(
                            psb, trend[:, k, j * 128:(j + 1) * 128], wds[:, k],
                            start=(k == 0), stop=(k == KC - 1),
                        )
                    if t == T - 1:
                        osb = obuf.tile([128, 1, D], dt.float16, tag="osb1", name="osb1")
                        nc.vector.scalar_tensor_tensor(
                            osb[:, 0], psb, 1.0, outa,
                            mybir.AluOpType.mult, mybir.AluOpType.add,
                        )
                        nc.scalar.dma_start(
                            out[j * 128:(j + 1) * 128, t:t + 1, :], osb)
                    else:
                        if t % 2 == 0:
                            osb = obuf.tile([128, 2, D], dt.float16, tag="osb", name="osb")
                            osb_cur[j] = osb
                        else:
                            osb = osb_cur[j]
                        nc.vector.scalar_tensor_tensor(
                            osb[:, t % 2], psb, 1.0, outa,
                            mybir.AluOpType.mult, mybir.AluOpType.add,
                        )
                        if t % 2 == 1:
                            nc.scalar.dma_start(
                                out[j * 128:(j + 1) * 128, t - 1:t + 1, :], osb)
    nc.compile()
    return nc


_NC_CACHE = {}


def _get_nc(mode):
    if mode not in _NC_CACHE:
        _NC_CACHE[mode] = build_v3() if mode == "v3" else build_bf16()
    return _NC_CACHE[mode]


def _to_pkt(w):  # [T, D, C] -> [128, T, KC, D]  (c = k*128 + p)
    wt = w.transpose(2, 0, 1).reshape(KC, 128, T, D)
    return np.ascontiguousarray(wt.transpose(1, 2, 0, 3))


def kernel(x, W_seasonal, b_seasonal, W_trend, b_trend, _trace=False):
    mode = MODE
    bf16 = ml_dtypes.bfloat16
    e4 = ml_dtypes.float8_e4m3
    nc = _get_nc(mode)

    bias = (b_seasonal + b_trend).reshape(1, T * D).astype(bf16)
    Wd = (W_trend - W_seasonal) / 37.0

    if mode == "v3":
        wst = _to_pkt(W_seasonal).astype(bf16)
        wdt = _to_pkt(Wd)
        wdb = np.ascontiguousarray(wdt[:, :, NDR:KC]).astype(bf16)
        wd8 = np.ascontiguousarray(wdt[:, :, 0:NDR] * SCL).astype(e4)
        in_maps = []
        for i in range(NCORES):
            xs = x[i * BC:(i + 1) * BC]                    # [BC, T, C]
            xti = xs.transpose(2, 1, 0).reshape(KC, 128, T, BC)
            xti = np.ascontiguousarray(
                xti.transpose(1, 2, 0, 3)).astype(bf16).reshape(128, T, KC * BC)
            in_maps.append({"xt": xti, "wst": wst, "wdb": wdb, "wd8": wd8,
                            "bias": bias})
    else:
        def to_tpkd(w):  # [T, D, C] -> [T, 128, KC, D]
            wt = w.transpose(0, 2, 1).reshape(T, KC, 128, D)
            return np.ascontiguousarray(wt.transpose(0, 2, 1, 3))

        wst = to_tpkd(W_seasonal).astype(bf16)
        wdt = to_tpkd(Wd).astype(bf16)
        in_maps = []
        for i in range(NCORES):
            xs = x[i * BC:(i + 1) * BC]
            xti = xs.transpose(1, 2, 0).reshape(T, KC, 128, BC)
            xti = np.ascontiguousarray(xti.transpose(0, 2, 1, 3)).astype(bf16)
            in_maps.append({"xt": xti, "wst": wst, "wdt": wdt, "bias": bias})

    res = run_bass_kernel_spmd(
        nc, in_maps, core_ids=list(range(NCORES)), trace=_trace
    )
    outp = np.concatenate([r["out"] for r in res.results], axis=0)
    if outp.dtype != np.float32:
        outp = outp.astype(np.float32)
    if _trace:
        return outp, res
    return outp


if __name__ == "__main__":
    rng = np.random.default_rng(0)
    x = rng.standard_normal((B, T, C), dtype=np.float32)
    Ws = rng.uniform(-0.04, 0.04, (T, D, C)).astype(np.float32)
    Wt = rng.uniform(-0.04, 0.04, (T, D, C)).astype(np.float32)
    bs = rng.uniform(-0.04, 0.04, (T, D)).astype(np.float32)
    bt = rng.uniform(-0.04, 0.04, (T, D)).astype(np.float32)
    o = kernel(x, Ws, bs, Wt, bt)
    print("out shape:", o.shape, o.dtype)
